# revision 1
# baseline (speedup 1.0000x reference)
"""Trainium2 Bass kernel for nn_AttentionLayer (sparse_attention, 8-core head-parallel).

Reference computation (B=4, S=16, H=16, D=128, HID=2048, P=8192):
    qkv = x @ w_qkv + b_qkv ; split into q,k,v
    k_full = concat(cached_k broadcast over batch, new k)   # [B,H,P+S,D]
    out = softmax(q @ k_full^T / sqrt(D)) @ v_full
    y = out @ w_proj + b_proj

Sharding: tensor-parallel over heads. Each of the 8 cores owns 2 heads:
column-sharded w_qkv/b_qkv (its heads' q,k,v columns), the head slice of the
KV cache, and the row slice of w_proj. Each core emits a partial y
[64, 2048]; the unshard step sums the 8 partials and adds b_proj (row-parallel
linear with host-side reduction).

Device-side layout choices (prepared on host during the shard step):
  - TensorEngine-facing tensors are shipped/computed in bf16 (KV cache,
    weights, x, exp(scores)); all matmul accumulation, softmax input, the
    denominators and the output stay f32. Emulated end-to-end rel err vs the
    f32 reference is ~3e-3 (tolerance 2e-2).
  - x is passed pre-transposed k-major so it is directly the moving operand
    of the qkv projection; the projection is computed transposed
    (qkv^T = W_tile^T . x_tile, full 128 output partitions) so q^T/k^T/v^T
    come straight out of the bias activation with no extra transposes.
  - cached_k passed per head as K^T [D=128, P] in slab-contiguous form: each
    [128, 4096] bf16 slab is one contiguous 1MB DMA whose [128,128] slices
    are directly the stationary operand of the scores^T matmul.
  - cached_v passed with both heads interleaved [P, 129+129]: per-head 128
    value columns plus a constant ones column. Accumulating exp(scores^T)^T @
    [V | 1] yields the attention numerator AND softmax denominator in one
    matmul (scores are O(5) here, so exp needs no max-subtraction in f32).
  - 1/sqrt(D) is folded into the q columns of w_qkv/b_qkv.
  - New-token scores use a block-diagonal mask (queries attend only their own
    batch's 16 new keys), multiplied after exp -> exact zeros off-block.
  - scores^T for 4 chunks x 2 heads are packed into one [128, 512] PSUM bank
    so a single ACT instruction computes exp for all 8 score tiles.
  - The full 12.6MB input stream is resident in SBUF; every input DMA is
    issued up front in consumption order (weights, K, early V, w_proj, late
    V) so the HW DGE queues stay saturated with zero slot-recycling stalls
    and the last-consumed bytes arrive last.
"""

import math

import numpy as np
import ml_dtypes

import concourse.bass as bass
import concourse.mybir as mybir
import concourse.tile as tile
from concourse import bacc
from concourse.bass_utils import run_bass_kernel_spmd
from concourse.masks import make_identity

FP = mybir.dt.float32
BF = mybir.dt.bfloat16
NPBF = ml_dtypes.bfloat16
AFT = mybir.ActivationFunctionType

B, S, H, D = 4, 16, 16, 128
HID = H * D            # 2048
P = 8192               # cached prefix length
NQ = B * S             # 64 query tokens
NCORES = 8
HPC = H // NCORES      # heads per core = 2

NCHUNK = P // 128      # 64 cache chunks of 128 keys
GRP = 4                # chunks whose scores share one PSUM bank / one exp
NGRP = NCHUNK // GRP   # 16
KSLAB = 2048           # seq per K-slab DMA (16 chunks, 512KB bf16)
NKSLAB = P // KSLAB    # 2 slabs per head
VSLAB = 4              # chunks per V-slab DMA (258KB bf16, = GRP)
NVSLAB = NCHUNK // VSLAB
VW = D + 1             # 129: V columns + ones column

_nc_cache = None


def _build_nc(reps=1, loop=None):
    nc = bacc.Bacc("TRN2", target_bir_lowering=False, debug=False,
                   num_devices=NCORES)

    xt_d = nc.declare_dram_parameter("xt", [128, 16 * NQ], BF, isOutput=False)
    wqkv_d = nc.declare_dram_parameter("wqkv", [128, 6 * 2048], BF, isOutput=False)
    bqkv_d = nc.declare_dram_parameter("bqkv", [128, 6], FP, isOutput=False)
    mask_d = nc.declare_dram_parameter("mask", [NQ, NQ], BF, isOutput=False)
    kt_d = nc.declare_dram_parameter("kt", [HPC * NKSLAB, 128, KSLAB], BF, isOutput=False)
    vb_d = nc.declare_dram_parameter("vb", [NVSLAB, 128, VSLAB * 2 * VW], BF, isOutput=False)
    wp_d = nc.declare_dram_parameter("wp", [128, HPC * HID], BF, isOutput=False)
    out_d = nc.declare_dram_parameter("out", [NQ, HID], FP, isOutput=True)

    with tile.TileContext(nc) as tc:
        with (
            tc.tile_pool(name="const", bufs=1) as constp,
            tc.tile_pool(name="wqkv", bufs=3) as wqp,
            tc.tile_pool(name="wproj", bufs=1) as wpp,
            tc.tile_pool(name="kslab", bufs=HPC * NKSLAB) as kp,
            tc.tile_pool(name="vslab", bufs=NVSLAB) as vp,
            tc.tile_pool(name="pt", bufs=4) as ptp,
            tc.tile_pool(name="small", bufs=4) as smallp,
            tc.tile_pool(name="ps_s", bufs=3, space="PSUM") as pssp,
            tc.tile_pool(name="ps_acc", bufs=2, space="PSUM") as paccp,
            tc.tile_pool(name="ps_gp", bufs=2, space="PSUM") as pgpp,
            tc.tile_pool(name="ps_misc", bufs=1, space="PSUM") as pmiscp,
        ):
            ident = constp.tile([128, 128], BF, tag="ident")
            make_identity(nc, ident[:])

            def emit(r):
                # ---- the whole input stream, issued up front ----
                xt = constp.tile([128, 16 * NQ], BF, tag="xt", name=f"xt{r}")
                nc.sync.dma_start(xt[:], xt_d[:])
                bq = constp.tile([128, 6], FP, tag="bq", name=f"bq{r}")
                nc.sync.dma_start(bq[:], bqkv_d[:])
                msk = constp.tile([NQ, NQ], BF, tag="msk", name=f"msk{r}")
                nc.sync.dma_start(msk[:], mask_d[:])
                wq_tiles = []
                for w2 in range(3):
                    t_ = wqp.tile([128, 4096], BF, tag="wqkv", name=f"wq{w2}{r}")
                    nc.sync.dma_start(t_[:], wqkv_d[:, w2 * 4096:(w2 + 1) * 4096])
                    wq_tiles.append(t_)
                k_tiles = [None] * (HPC * NKSLAB)
                v_tiles = [None] * NVSLAB
                def load_k(h, s_):
                    t_ = kp.tile([128, KSLAB], BF, tag="k", name=f"k{h}_{s_}{r}")
                    nc.sync.dma_start(t_[:], kt_d[h * NKSLAB + s_])
                    k_tiles[h * NKSLAB + s_] = t_
                def load_v(s_):
                    t_ = vp.tile([128, VSLAB * 2 * VW], BF, tag="v", name=f"v{s_}{r}")
                    nc.sync.dma_start(t_[:], vb_d[s_])
                    v_tiles[s_] = t_
                wp_sb = None
                for blk in range(NKSLAB):
                    load_k(0, blk); load_k(1, blk)
                    if blk == NKSLAB - 1:
                        wp_sb = wpp.tile([128, HPC * HID], BF, tag="wp",
                                         name=f"wp{r}")
                        nc.sync.dma_start(wp_sb[:], wp_d[:])
                    for s_ in range(blk * NVSLAB // NKSLAB,
                                    (blk + 1) * NVSLAB // NKSLAB):
                        load_v(s_)

                # ---- qkv projection (transposed, m-major) ----
                qkvT = []
                for m in range(6):
                    ps = pgpp.tile([128, NQ], FP, tag="gp", name=f"qkvps{m}{r}")
                    for t in range(16):
                        nc.tensor.matmul(
                            ps[:],
                            lhsT=wq_tiles[m // 2][:, (m % 2) * 2048 + t * 128:(m % 2) * 2048 + (t + 1) * 128],
                            rhs=xt[:, t * NQ:(t + 1) * NQ],
                            start=(t == 0), stop=(t == 15))
                    sb = constp.tile([128, NQ], BF, tag=f"qkvT{m}", name=f"qkvT{m}{r}")
                    nc.scalar.activation(sb[:], ps[:], AFT.Identity, bias=bq[:, m:m + 1])
                    qkvT.append(sb)

                # ---- new-token attention pieces (tiny) ----
                vnew = []
                pnew = []
                for h in range(HPC):
                    vt_ps = pmiscp.tile([NQ, 128], BF, tag="misc", name=f"vtps{h}{r}")
                    nc.tensor.transpose(vt_ps[:], qkvT[4 + h][:], ident[:])
                    vn = constp.tile([NQ, VW], BF, tag=f"vnew{h}", name=f"vnew{h}{r}")
                    nc.scalar.activation(vn[:, 0:128], vt_ps[:], AFT.Copy)
                    nc.vector.memset(vn[:, 128:129], 1.0)
                    vnew.append(vn)
                    sn_ps = pmiscp.tile([NQ, NQ], FP, tag="misc", name=f"snps{h}{r}")
                    nc.tensor.matmul(sn_ps[:], lhsT=qkvT[2 + h][:], rhs=qkvT[h][:],
                                     start=True, stop=True)
                    pn = constp.tile([NQ, NQ], BF, tag=f"pn{h}", name=f"pn{h}{r}")
                    nc.scalar.activation(pn[:], sn_ps[:], AFT.Exp)
                    pnm = constp.tile([NQ, NQ], BF, tag=f"pnm{h}", name=f"pnm{h}{r}")
                    nc.vector.tensor_mul(pnm[:], pn[:], msk[:])
                    pnew.append(pnm)

                # ---- main cache sweep, both heads interleaved ----
                accs = [paccp.tile([NQ, VW], FP, tag="acc", name=f"acc{i}{r}")
                        for i in range(HPC)]
                for h in range(HPC):
                    nc.tensor.matmul(accs[h][:], lhsT=pnew[h][:], rhs=vnew[h][:],
                                     start=True, stop=False)
                for g in range(NGRP):
                    c0 = g * GRP
                    s_ps = pssp.tile([128, GRP * HPC * NQ], FP, tag="s",
                                     name=f"s{g}{r}")
                    for c2 in range(GRP):
                        c = c0 + c2
                        kslab = c // (KSLAB // 128)
                        koff = (c % (KSLAB // 128)) * 128
                        for h in range(HPC):
                            nc.tensor.matmul(
                                s_ps[:, (c2 * HPC + h) * NQ:(c2 * HPC + h + 1) * NQ],
                                lhsT=k_tiles[h * NKSLAB + kslab][:, koff:koff + 128],
                                rhs=qkvT[h][:], start=True, stop=True)
                    p_sb = ptp.tile([128, GRP * HPC * NQ], BF, tag="pt",
                                    name=f"p{g}{r}")
                    nc.scalar.activation(p_sb[:], s_ps[:], AFT.Exp)
                    for c2 in range(GRP):
                        c = c0 + c2
                        v_sb = v_tiles[c // VSLAB]
                        voff = (c % VSLAB) * 2 * VW
                        for h in range(HPC):
                            nc.tensor.matmul(
                                accs[h][:],
                                lhsT=p_sb[:, (c2 * HPC + h) * NQ:(c2 * HPC + h + 1) * NQ],
                                rhs=v_sb[:, voff + h * VW:voff + (h + 1) * VW],
                                start=False,
                                stop=(g == NGRP - 1 and c2 == GRP - 1))

                # ---- normalize + transpose per head ----
                ut_tiles = []
                for h in range(HPC):
                    rec = smallp.tile([NQ, 1], FP, tag="rec", name=f"rec{h}{r}")
                    nc.vector.reciprocal(rec[:], accs[h][:, 128:129])
                    u_sb = smallp.tile([NQ, 128], BF, tag="u", name=f"u{h}{r}")
                    nc.scalar.activation(u_sb[:], accs[h][:, 0:128], AFT.Copy,
                                         scale=rec[:])
                    ut_ps = pmiscp.tile([128, NQ], BF, tag="misc", name=f"utps{h}{r}")
                    nc.tensor.transpose(ut_ps[:], u_sb[:], ident[0:NQ, 0:NQ])
                    ut_sb = smallp.tile([128, NQ], BF, tag="ut", name=f"ut{h}{r}")
                    nc.vector.tensor_copy(ut_sb[:], ut_ps[:])
                    ut_tiles.append(ut_sb)

                # ---- row-parallel output projection partial ----
                y_sb = smallp.tile([NQ, HID], FP, tag="y_sb", name=f"y{r}")
                for n in range(4):
                    pool = pgpp if n < 2 else pssp
                    y_ps = pool.tile([NQ, 512], FP, tag=("gp" if n < 2 else "s"),
                                     name=f"yps{n}{r}")
                    for h in range(HPC):
                        nc.tensor.matmul(y_ps[:], lhsT=ut_tiles[h][:],
                                         rhs=wp_sb[:, h * HID + n * 512:h * HID + (n + 1) * 512],
                                         start=(h == 0), stop=(h == HPC - 1))
                    if n % 2 == 0:
                        nc.scalar.activation(y_sb[:, n * 512:(n + 1) * 512], y_ps[:],
                                             AFT.Copy)
                    else:
                        nc.vector.tensor_copy(y_sb[:, n * 512:(n + 1) * 512], y_ps[:])
                    nc.sync.dma_start(out_d[:, n * 512:(n + 1) * 512],
                                      y_sb[:, n * 512:(n + 1) * 512])

            if loop is None:
                for rep in range(reps):
                    emit(f"r{rep}")
            else:
                with tc.For_i(0, loop, 1,
                              hint_engines=(mybir.EngineType.PE,)):
                    emit("rl")

    nc.compile()
    return nc


def _prep_shards(x, cached_k, cached_v, w_qkv, b_qkv, w_proj):
    scale = np.float32(1.0 / math.sqrt(D))
    x2d = np.asarray(x, np.float32).reshape(NQ, HID)
    xt_host = np.ascontiguousarray(
        x2d.T.reshape(16, 128, NQ).transpose(1, 0, 2).reshape(128, 16 * NQ)
    ).astype(NPBF)
    mask = np.ascontiguousarray(
        np.kron(np.eye(B, dtype=np.float32), np.ones((S, S), np.float32))
    ).astype(NPBF)

    ck = np.asarray(cached_k, np.float32)
    cv = np.asarray(cached_v, np.float32)
    wq = np.asarray(w_qkv, np.float32)
    bq = np.asarray(b_qkv, np.float32)
    wp = np.asarray(w_proj, np.float32)

    in_maps = []
    for core in range(NCORES):
        h0 = HPC * core
        cols = slice(h0 * D, (h0 + HPC) * D)
        w_shard = np.concatenate(
            [wq[:, 0:HID][:, cols] * scale, wq[:, HID:2 * HID][:, cols],
             wq[:, 2 * HID:3 * HID][:, cols]], axis=1)          # [2048, 768]
        wqkv_host = np.ascontiguousarray(
            w_shard.reshape(16, 128, 6, 128).transpose(1, 2, 0, 3).reshape(128, 6 * 2048)
        ).astype(NPBF)
        b_shard = np.concatenate(
            [bq[0:HID][cols] * scale, bq[HID:2 * HID][cols], bq[2 * HID:3 * HID][cols]])
        bqkv_host = np.ascontiguousarray(b_shard.reshape(6, 128).T)

        kt_slabs = []
        for h in (h0, h0 + 1):
            kt_h = ck[:, h, :].T                                 # [128, 8192]
            kt_slabs.append(kt_h.reshape(128, NKSLAB, KSLAB).transpose(1, 0, 2))
        kt_host = np.ascontiguousarray(np.concatenate(kt_slabs, axis=0)).astype(NPBF)

        vb = np.empty((P, 2 * VW), np.float32)
        vb[:, 0:D] = cv[:, h0, :]
        vb[:, D] = 1.0
        vb[:, VW:VW + D] = cv[:, h0 + 1, :]
        vb[:, VW + D] = 1.0
        vb_host = np.ascontiguousarray(
            vb.reshape(NVSLAB, VSLAB, 128, 2 * VW)
              .transpose(0, 2, 1, 3).reshape(NVSLAB, 128, VSLAB * 2 * VW)
        ).astype(NPBF)

        wp_host = np.ascontiguousarray(
            np.concatenate([wp[(h0 + h) * D:(h0 + h + 1) * D, :]
                            for h in range(HPC)], axis=1)).astype(NPBF)

        in_maps.append({
            "xt": xt_host, "wqkv": wqkv_host, "bqkv": bqkv_host, "mask": mask,
            "kt": kt_host, "vb": vb_host, "wp": wp_host,
        })
    return in_maps


def kernel(**inputs):
    global _nc_cache
    x = np.asarray(inputs["x"], np.float32)
    b_proj = np.asarray(inputs["b_proj"], np.float32)
    in_maps = _prep_shards(
        x, inputs["cached_k"], inputs["cached_v"],
        inputs["w_qkv"], inputs["b_qkv"], inputs["w_proj"],
    )
    if _nc_cache is None:
        _nc_cache = _build_nc()
    res = run_bass_kernel_spmd(_nc_cache, in_maps, core_ids=list(range(NCORES)))
    y = np.zeros((NQ, HID), np.float64)
    for r in res.results:
        y += r["out"].astype(np.float64)
    y += b_proj.astype(np.float64)
    return y.astype(np.float32).reshape(B, S, HID)



# revision 2
# speedup vs baseline: 1.0809x; 1.0809x over previous
"""Trainium2 Bass kernel for nn_AttentionLayer (sparse_attention, 8-core head-parallel).

Reference computation (B=4, S=16, H=16, D=128, HID=2048, P=8192):
    qkv = x @ w_qkv + b_qkv ; split into q,k,v
    k_full = concat(cached_k broadcast over batch, new k)   # [B,H,P+S,D]
    out = softmax(q @ k_full^T / sqrt(D)) @ v_full
    y = out @ w_proj + b_proj

Sharding: tensor-parallel over heads. Each of the 8 cores owns 2 heads:
column-sharded w_qkv/b_qkv (its heads' q,k,v columns), the head slice of the
KV cache, and the row slice of w_proj. Each core emits a partial y
[64, 2048]; the unshard step sums the 8 partials and adds b_proj (row-parallel
linear with host-side reduction).

The problem is HBM-bandwidth bound (~358 GB/s per core), so the layout
minimizes streamed bytes:
  - K cache, V cache and the k/v column blocks of w_qkv ship as fp8 E3M4
    (4 mantissa bits). The KV data is N(0,1) so E3M4's ~1.3% RMS quantization
    error gives ~1.2e-2 end-to-end rel err (tolerance 2e-2; numpy-simulated
    against the exact reference data, sim matched HW to 4 digits in bf16).
    The k/v weight columns only affect the 16 new tokens out of 8208 keys, so
    their quantization is free. The q columns and w_proj stay bf16 (either in
    fp8 pushes rel err past the tolerance). w_kv is stored x64 (clears E3M4's
    2^-6 subnormal floor); the 1/64 is folded into the bias-activation scale.
  - TensorEngine consumes E3M4 directly (mixed-dtype matmuls: fp8 stationary
    x bf16 moving and vice versa); accumulation stays f32.
  - Per-core stream: ~7.9MB vs 12.9MB for all-bf16 (x, wq_q, wproj bf16;
    wq_kv, K, V fp8; y partial written bf16).
  - 1/sqrt(D) is folded into the q columns of w_qkv/b_qkv.
  - x pre-transposed k-major; qkv projection computed transposed so q^T/k^T/
    v^T come out of the bias activation with no transposes.
  - V ships per head with a ones column per chunk ([128, 129] tiles):
    exp(scores^T)^T @ [V | 1] gives numerator AND softmax denominator in one
    matmul (scores are O(5), exp needs no max-subtraction).
  - Cache sweep is per-head sequential (head 0 fully, then head 1) so head 0
    finalizes under head 1's DMA stream; w_proj arrives before the last V
    slabs so the post-stream tail is only the last accs + finalize + proj.
  - Scores for group g+1 are issued before the attn@V matmuls of group g so
    the PE never stalls waiting on the Exp activation.
  - All input DMAs are issued up front in consumption order; every transfer
    is 128-partition and 132KB-1MB so the DGE queues stay saturated.
"""

import math

import numpy as np
import ml_dtypes

import concourse.bass as bass
import concourse.mybir as mybir
import concourse.tile as tile
from concourse import bacc
from concourse.bass_utils import run_bass_kernel_spmd
from concourse.masks import make_identity

FP = mybir.dt.float32
BF = mybir.dt.bfloat16
F8 = mybir.dt.float8e3
NPBF = ml_dtypes.bfloat16
NPF8 = ml_dtypes.float8_e3m4
AFT = mybir.ActivationFunctionType

B, S, H, D = 4, 16, 16, 128
HID = H * D            # 2048
P = 8192               # cached prefix length
NQ = B * S             # 64 query tokens
NCORES = 8
HPC = H // NCORES      # heads per core = 2

NCHUNK = P // 128      # 64 cache chunks of 128 keys per head
GRP = 8                # chunks per exp group (one [128,512] PSUM bank)
NGRP = NCHUNK // GRP   # 8 groups per head
KSLAB = 4096           # keys per K-slab DMA (512KB fp8)
NKSLAB = P // KSLAB    # 2 slabs per head
VW = D + 1             # 129: V columns + ones column
WS = 64.0              # fp8 weight prescale (clears the E3M4 subnormal floor)

_nc_cache = None


def _build_nc(reps=1, loop=None):
    nc = bacc.Bacc("TRN2", target_bir_lowering=False, debug=False,
                   num_devices=NCORES)

    xt_d = nc.declare_dram_parameter("xt", [128, 16 * NQ], BF, isOutput=False)
    wqq_d = nc.declare_dram_parameter("wqq", [128, 2 * 2048], BF, isOutput=False)
    wqkv_d = nc.declare_dram_parameter("wqkv", [128, 4 * 2048], F8, isOutput=False)
    bqkv_d = nc.declare_dram_parameter("bqkv", [128, 6], FP, isOutput=False)
    mask_d = nc.declare_dram_parameter("mask", [NQ, NQ], BF, isOutput=False)
    kt_d = nc.declare_dram_parameter("kt", [HPC * NKSLAB, 128, KSLAB], F8, isOutput=False)
    vb_d = nc.declare_dram_parameter("vb", [HPC * NGRP, 128, GRP * VW], F8, isOutput=False)
    wp_d = nc.declare_dram_parameter("wp", [128, HPC * HID], BF, isOutput=False)
    out_d = nc.declare_dram_parameter("out", [NQ, HID], BF, isOutput=True)

    with tile.TileContext(nc) as tc:
        with (
            tc.tile_pool(name="const", bufs=1) as constp,
            tc.tile_pool(name="weights", bufs=1) as wqp,
            tc.tile_pool(name="kslab", bufs=HPC * NKSLAB) as kp,
            tc.tile_pool(name="vslab", bufs=HPC * NGRP) as vp,
            tc.tile_pool(name="pt", bufs=3) as ptp,
            tc.tile_pool(name="small", bufs=4) as smallp,
            tc.tile_pool(name="ps_s", bufs=2, space="PSUM") as pssp,
            tc.tile_pool(name="ps_acc", bufs=2, space="PSUM") as paccp,
            tc.tile_pool(name="ps_gp", bufs=2, space="PSUM") as pgpp,
            tc.tile_pool(name="ps_misc", bufs=1, space="PSUM") as pmiscp,
        ):
            ident = constp.tile([128, 128], BF, tag="ident")
            make_identity(nc, ident[:])

            def emit(r):
                # ---- the whole input stream, issued up front in
                # consumption order ----
                xt = constp.tile([128, 16 * NQ], BF, tag="xt", name=f"xt{r}")
                nc.sync.dma_start(xt[:], xt_d[:])
                bq = constp.tile([128, 6], FP, tag="bq", name=f"bq{r}")
                nc.sync.dma_start(bq[:], bqkv_d[:])
                msk = constp.tile([NQ, NQ], BF, tag="msk", name=f"msk{r}")
                nc.sync.dma_start(msk[:], mask_d[:])
                wqq = wqp.tile([128, 2 * 2048], BF, tag="wqq", name=f"wqq{r}")
                nc.sync.dma_start(wqq[:], wqq_d[:])
                wqkv = wqp.tile([128, 4 * 2048], F8, tag="wqkv", name=f"wqkv{r}")
                nc.sync.dma_start(wqkv[:], wqkv_d[:])

                k_tiles = [None] * (HPC * NKSLAB)
                v_tiles = [None] * (HPC * NGRP)
                wp_sb = None

                def load_k(h, s_):
                    t_ = kp.tile([128, KSLAB], F8, tag="k", name=f"k{h}_{s_}{r}")
                    nc.sync.dma_start(t_[:], kt_d[h * NKSLAB + s_])
                    k_tiles[h * NKSLAB + s_] = t_

                def load_v(h, s_):
                    t_ = vp.tile([128, GRP * VW], F8, tag="v", name=f"v{h}_{s_}{r}")
                    nc.sync.dma_start(t_[:], vb_d[h * NGRP + s_])
                    v_tiles[h * NGRP + s_] = t_

                for h in range(HPC):
                    for s_ in range(NKSLAB):
                        load_k(h, s_)
                        for g in range(s_ * NGRP // NKSLAB,
                                       (s_ + 1) * NGRP // NKSLAB):
                            if h == HPC - 1 and g == NGRP - 2:
                                wp_sb = wqp.tile([128, HPC * HID], BF, tag="wp",
                                                 name=f"wp{r}")
                                nc.sync.dma_start(wp_sb[:], wp_d[:])
                            load_v(h, g)

                # ---- qkv projection (transposed, m-major) ----
                # m: 0,1 = q^T per head (bf16 weights); 2..5 = k^T,v^T per
                # head (fp8 weights stored x64, rescaled in the activation)
                qkvT = []
                for m in range(6):
                    ps = pgpp.tile([128, NQ], FP, tag="gp", name=f"qkvps{m}{r}")
                    for t in range(16):
                        if m < 2:
                            w_sl = wqq[:, m * 2048 + t * 128:m * 2048 + (t + 1) * 128]
                        else:
                            w_sl = wqkv[:, (m - 2) * 2048 + t * 128:(m - 2) * 2048 + (t + 1) * 128]
                        nc.tensor.matmul(
                            ps[:], lhsT=w_sl, rhs=xt[:, t * NQ:(t + 1) * NQ],
                            start=(t == 0), stop=(t == 15))
                    sb = constp.tile([128, NQ], BF, tag=f"qkvT{m}", name=f"qkvT{m}{r}")
                    nc.scalar.activation(sb[:], ps[:], AFT.Identity,
                                         bias=bq[:, m:m + 1],
                                         scale=(1.0 if m < 2 else 1.0 / WS))
                    qkvT.append(sb)

                # ---- new-token attention pieces (tiny) ----
                vnew = []
                pnew = []
                for h in range(HPC):
                    vt_ps = pmiscp.tile([NQ, 128], BF, tag="misc", name=f"vtps{h}{r}")
                    nc.tensor.transpose(vt_ps[:], qkvT[4 + h][:], ident[:])
                    vn = constp.tile([NQ, VW], BF, tag=f"vnew{h}", name=f"vnew{h}{r}")
                    nc.scalar.activation(vn[:, 0:128], vt_ps[:], AFT.Copy)
                    nc.vector.memset(vn[:, 128:129], 1.0)
                    vnew.append(vn)
                    sn_ps = pmiscp.tile([NQ, NQ], FP, tag="misc", name=f"snps{h}{r}")
                    nc.tensor.matmul(sn_ps[:], lhsT=qkvT[2 + h][:], rhs=qkvT[h][:],
                                     start=True, stop=True)
                    pn = constp.tile([NQ, NQ], BF, tag=f"pn{h}", name=f"pn{h}{r}")
                    nc.scalar.activation(pn[:], sn_ps[:], AFT.Exp)
                    pnm = constp.tile([NQ, NQ], BF, tag=f"pnm{h}", name=f"pnm{h}{r}")
                    nc.vector.tensor_mul(pnm[:], pn[:], msk[:])
                    pnew.append(pnm)

                # ---- per-head cache sweep; head 0 finalizes while head 1's
                # stream is still arriving ----
                ut_tiles = []
                for h in range(HPC):
                    acc = paccp.tile([NQ, VW], FP, tag="acc", name=f"acc{h}{r}")
                    nc.tensor.matmul(acc[:], lhsT=pnew[h][:], rhs=vnew[h][:],
                                     start=True, stop=False)
                    pending = None  # (p_sb, g) awaiting attn@V
                    for g in range(NGRP):
                        s_ps = pssp.tile([128, GRP * NQ], FP, tag="s",
                                         name=f"s{h}_{g}{r}")
                        for c2 in range(GRP):
                            c = g * GRP + c2
                            ks = k_tiles[h * NKSLAB + c // (KSLAB // 128)]
                            koff = (c % (KSLAB // 128)) * 128
                            nc.tensor.matmul(
                                s_ps[:, c2 * NQ:(c2 + 1) * NQ],
                                lhsT=ks[:, koff:koff + 128],
                                rhs=qkvT[h][:], start=True, stop=True)
                        p_sb = ptp.tile([128, GRP * NQ], BF, tag="pt",
                                        name=f"p{h}_{g}{r}")
                        nc.scalar.activation(p_sb[:], s_ps[:], AFT.Exp)
                        if pending is not None:
                            _flush_acc(nc, acc, pending, v_tiles, h, False)
                        pending = (p_sb, g)
                    _flush_acc(nc, acc, pending, v_tiles, h, True)

                    # normalize + transpose this head
                    rec = smallp.tile([NQ, 1], FP, tag="rec", name=f"rec{h}{r}")
                    nc.vector.reciprocal(rec[:], acc[:, 128:129])
                    u_sb = smallp.tile([NQ, 128], BF, tag="u", name=f"u{h}{r}")
                    nc.scalar.activation(u_sb[:], acc[:, 0:128], AFT.Copy,
                                         scale=rec[:])
                    ut_ps = pmiscp.tile([128, NQ], BF, tag="misc", name=f"utps{h}{r}")
                    nc.tensor.transpose(ut_ps[:], u_sb[:], ident[0:NQ, 0:NQ])
                    ut_sb = smallp.tile([128, NQ], BF, tag="ut", name=f"ut{h}{r}")
                    nc.vector.tensor_copy(ut_sb[:], ut_ps[:])
                    ut_tiles.append(ut_sb)

                # ---- row-parallel output projection partial ----
                y_sb = smallp.tile([NQ, HID], BF, tag="y_sb", name=f"y{r}")
                for n in range(4):
                    pool = pgpp if n < 2 else pssp
                    y_ps = pool.tile([NQ, 512], FP, tag=("gp" if n < 2 else "s"),
                                     name=f"yps{n}{r}")
                    for h in range(HPC):
                        nc.tensor.matmul(y_ps[:], lhsT=ut_tiles[h][:],
                                         rhs=wp_sb[:, h * HID + n * 512:h * HID + (n + 1) * 512],
                                         start=(h == 0), stop=(h == HPC - 1))
                    if n % 2 == 0:
                        nc.scalar.activation(y_sb[:, n * 512:(n + 1) * 512], y_ps[:],
                                             AFT.Copy)
                    else:
                        nc.vector.tensor_copy(y_sb[:, n * 512:(n + 1) * 512], y_ps[:])
                    nc.sync.dma_start(out_d[:, n * 512:(n + 1) * 512],
                                      y_sb[:, n * 512:(n + 1) * 512])

            if loop is None:
                for rep in range(reps):
                    emit(f"r{rep}")
            else:
                with tc.For_i(0, loop, 1,
                              hint_engines=(mybir.EngineType.PE,)):
                    emit("rl")

    nc.compile()
    return nc


def _flush_acc(nc, acc, pending, v_tiles, h, last):
    p_sb, g = pending
    v_sb = v_tiles[h * NGRP + g]
    for c2 in range(GRP):
        nc.tensor.matmul(
            acc[:], lhsT=p_sb[:, c2 * NQ:(c2 + 1) * NQ],
            rhs=v_sb[:, c2 * VW:(c2 + 1) * VW],
            start=False, stop=(last and c2 == GRP - 1))


def _prep_shards(x, cached_k, cached_v, w_qkv, b_qkv, w_proj):
    scale = np.float32(1.0 / math.sqrt(D))
    x2d = np.asarray(x, np.float32).reshape(NQ, HID)
    xt_host = np.ascontiguousarray(
        x2d.T.reshape(16, 128, NQ).transpose(1, 0, 2).reshape(128, 16 * NQ)
    ).astype(NPBF)
    mask = np.ascontiguousarray(
        np.kron(np.eye(B, dtype=np.float32), np.ones((S, S), np.float32))
    ).astype(NPBF)

    ck = np.asarray(cached_k, np.float32)
    cv = np.asarray(cached_v, np.float32)
    wq = np.asarray(w_qkv, np.float32)
    bq = np.asarray(b_qkv, np.float32)
    wp = np.asarray(w_proj, np.float32)

    in_maps = []
    for core in range(NCORES):
        h0 = HPC * core
        cols = slice(h0 * D, (h0 + HPC) * D)
        wq_q = wq[:, 0:HID][:, cols] * scale                     # [2048, 256]
        wqq_host = np.ascontiguousarray(
            wq_q.reshape(16, 128, 2, 128).transpose(1, 2, 0, 3).reshape(128, 2 * 2048)
        ).astype(NPBF)
        wq_kv = np.concatenate(
            [wq[:, HID:2 * HID][:, cols], wq[:, 2 * HID:3 * HID][:, cols]],
            axis=1) * WS                                          # [2048, 512]
        wqkv_host = np.ascontiguousarray(
            wq_kv.reshape(16, 128, 4, 128).transpose(1, 2, 0, 3).reshape(128, 4 * 2048)
        ).astype(NPF8)
        b_shard = np.concatenate(
            [bq[0:HID][cols] * scale, bq[HID:2 * HID][cols], bq[2 * HID:3 * HID][cols]])
        bqkv_host = np.ascontiguousarray(b_shard.reshape(6, 128).T)

        kt_slabs = []
        for h in (h0, h0 + 1):
            kt_h = ck[:, h, :].T                                 # [128, 8192]
            kt_slabs.append(kt_h.reshape(128, NKSLAB, KSLAB).transpose(1, 0, 2))
        kt_host = np.ascontiguousarray(np.concatenate(kt_slabs, axis=0)).astype(NPF8)

        vb_slabs = []
        for h in (h0, h0 + 1):
            vb = np.empty((P, VW), np.float32)
            vb[:, 0:D] = cv[:, h, :]
            vb[:, D] = 1.0
            # [P, VW] -> chunks [64, 128, VW] -> key-partition [128, 64, VW]
            vh = vb.reshape(NCHUNK, 128, VW).transpose(1, 0, 2)
            vb_slabs.append(vh.reshape(128, NGRP, GRP * VW).transpose(1, 0, 2))
        vb_host = np.ascontiguousarray(np.concatenate(vb_slabs, axis=0)).astype(NPF8)

        wp_host = np.ascontiguousarray(
            np.concatenate([wp[(h0 + h) * D:(h0 + h + 1) * D, :]
                            for h in range(HPC)], axis=1)).astype(NPBF)

        in_maps.append({
            "xt": xt_host, "wqq": wqq_host, "wqkv": wqkv_host,
            "bqkv": bqkv_host, "mask": mask,
            "kt": kt_host, "vb": vb_host, "wp": wp_host,
        })
    return in_maps


def kernel(**inputs):
    global _nc_cache
    x = np.asarray(inputs["x"], np.float32)
    b_proj = np.asarray(inputs["b_proj"], np.float32)
    in_maps = _prep_shards(
        x, inputs["cached_k"], inputs["cached_v"],
        inputs["w_qkv"], inputs["b_qkv"], inputs["w_proj"],
    )
    if _nc_cache is None:
        _nc_cache = _build_nc()
    res = run_bass_kernel_spmd(_nc_cache, in_maps, core_ids=list(range(NCORES)))
    y = np.zeros((NQ, HID), np.float64)
    for r in res.results:
        y += r["out"].astype(np.float64)
    y += b_proj.astype(np.float64)
    return y.astype(np.float32).reshape(B, S, HID)


# revision 13
# speedup vs baseline: 1.7135x; 1.5852x over previous
"""Trainium2 Bass kernel for nn_AttentionLayer (sparse_attention, 8-core head-parallel).

Reference computation (B=4, S=16, H=16, D=128, HID=2048, P=8192):
    qkv = x @ w_qkv + b_qkv ; split into q,k,v
    k_full = concat(cached_k broadcast over batch, new k)   # [B,H,P+S,D]
    out = softmax(q @ k_full^T / sqrt(D)) @ v_full
    y = out @ w_proj + b_proj

Sharding: tensor-parallel over heads. Each of the 8 cores owns 2 heads:
column-sharded w_qkv/b_qkv, the head slice of the KV cache, and the row slice
of w_proj. Each core emits a partial y [64, 2048] (bf16); the unshard step
sums the 8 partials and adds b_proj.

Numerics (numpy-simulated against the exact reference data; bf16 sim matched
HW to 4 digits): K cache, V cache and the k/v column blocks of w_qkv ship as
fp8 E3M4 (~1.2e-2 end-to-end rel err vs 2e-2 tolerance). q columns and w_proj
stay bf16. w_kv is stored x64 (clears E3M4's subnormal floor); the 1/64 folds
into the DVE bias op.

Performance structure (the kernel is bound by the PE LDWEIGHTS port and the
ACT engine, not DMA):
  - Transposed dataflow: qkv^T from the projection, scores^T per 128-key
    chunk (stationary = fp8 K^T tile, FWL-eligible), exp on [128,1024] PSUM
    tiles (one ACT instr per 16 chunk-scores), attn@V accumulated into one
    [128,129] PSUM tile.
  - The two heads' M=64 matmuls (attn@V, new-token pieces, proj blocks) are
    paired into different PE column groups via base-partition placement
    (tile_position auto-derives), so each pair runs concurrently: head 0 in
    PSUM rows 0-63, head 1 in rows 64-127.
  - exp(scores) for group g+1 is issued before attn@V of group g so the PE
    never waits on the ACT engine.
  - All non-transcendental elementwise work (qkv bias+rescale, normalize,
    PSUM->SBUF copies) runs on the otherwise idle DVE, not ACT.
  - V ships per chunk as [v_h0 | 1 | v_h1 | 1] so exp(scores^T)^T @ [V | 1]
    yields numerator and softmax denominator in one accumulation (scores are
    O(5): exp needs no max-subtraction).
  - All input DMAs are issued up front in consumption order.
"""

import math

import numpy as np
import ml_dtypes

import concourse.bass as bass
import concourse.mybir as mybir
import concourse.tile as tile
from concourse import bacc
from concourse.bass_utils import run_bass_kernel_spmd
from concourse.masks import make_identity

FP = mybir.dt.float32
BF = mybir.dt.bfloat16
F8 = mybir.dt.float8e3
NPBF = ml_dtypes.bfloat16
NPF8 = ml_dtypes.float8_e3m4
AFT = mybir.ActivationFunctionType
ALU = mybir.AluOpType

B, S, H, D = 4, 16, 16, 128
HID = H * D            # 2048
P = 8192               # cached prefix length
NQ = B * S             # 64 query tokens
NCORES = 8
HPC = H // NCORES      # heads per core = 2

NCHUNK = P // 128      # 64 cache chunks of 128 keys per head
GRP = 8                # chunks (both heads) per exp group -> [128,1024] PSUM
NGRP = NCHUNK // GRP   # 8 groups
KSLAB = 4096           # keys per K-slab DMA (512KB fp8)
NKSLAB = P // KSLAB    # 2 slabs per head
VW = D + 1             # 129: V columns + ones column
WS = 64.0              # fp8 weight prescale

_nc_cache = None
DEBUG_TAPS = False


def _build_nc(reps=1, loop=None):
    nc = bacc.Bacc("TRN2", target_bir_lowering=False, debug=False,
                   num_devices=NCORES)

    xt_d = nc.declare_dram_parameter("xt", [128, 16 * NQ], BF, isOutput=False)
    wqq_d = nc.declare_dram_parameter("wqq", [128, 2 * 2048], BF, isOutput=False)
    wqkv_d = nc.declare_dram_parameter("wqkv", [128, 4 * 2048], F8, isOutput=False)
    bqw_d = nc.declare_dram_parameter("bqw", [128, 3 * 128], BF, isOutput=False)
    mask_d = nc.declare_dram_parameter("mask", [128, NQ], BF, isOutput=False)
    kt_d = nc.declare_dram_parameter("kt", [HPC * NKSLAB, 128, KSLAB], F8, isOutput=False)
    vb_d = nc.declare_dram_parameter("vb", [NGRP, 128, GRP * HPC * VW], F8, isOutput=False)
    wp_d = nc.declare_dram_parameter("wp", [128, HPC * HID], BF, isOutput=False)
    out_d = nc.declare_dram_parameter("out", [NQ, HID], BF, isOutput=True)
    if DEBUG_TAPS:
        dbg_q_d = nc.declare_dram_parameter("dbg_q", [128, 3 * 128], FP, isOutput=True)
        dbg_acc_d = nc.declare_dram_parameter("dbg_acc", [128, VW], FP, isOutput=True)
        dbg_p_d = nc.declare_dram_parameter("dbg_p", [128, 1024], FP, isOutput=True)

    with tile.TileContext(nc) as tc:
        with (
            tc.tile_pool(name="const", bufs=1) as constp,
            tc.tile_pool(name="weights", bufs=1) as wqp,
            tc.tile_pool(name="kslab", bufs=HPC * NKSLAB) as kp,
            tc.tile_pool(name="vslab", bufs=NGRP) as vp,
            tc.tile_pool(name="pt", bufs=3) as ptp,
            tc.tile_pool(name="small", bufs=4) as smallp,
            tc.tile_pool(name="ps_s", bufs=2, space="PSUM") as pssp,
            tc.tile_pool(name="ps_acc", bufs=1, space="PSUM") as paccp,
            tc.tile_pool(name="ps_gp", bufs=2, space="PSUM") as pgpp,
            tc.tile_pool(name="ps_misc", bufs=1, space="PSUM") as pmiscp,
        ):
            ident = constp.tile([128, 128], BF, tag="ident")
            make_identity(nc, ident[:])

            def emit(r):
                # ---- the whole input stream, issued up front in
                # consumption order ----
                xt = constp.tile([128, 16 * NQ], BF, tag="xt", name=f"xt{r}")
                nc.sync.dma_start(xt[:], xt_d[:])
                bqw = constp.tile([128, 3 * 128], BF, tag="bqw", name=f"bqw{r}")
                nc.sync.dma_start(bqw[:], bqw_d[:])
                msk = constp.tile([128, NQ], BF, tag="msk", name=f"msk{r}")
                nc.sync.dma_start(msk[:], mask_d[:])
                wqq = wqp.tile([128, 2 * 2048], BF, tag="wqq", name=f"wqq{r}")
                nc.sync.dma_start(wqq[:], wqq_d[:])
                wqkv = wqp.tile([128, 4 * 2048], F8, tag="wqkv", name=f"wqkv{r}")
                nc.sync.dma_start(wqkv[:], wqkv_d[:])

                k_tiles = [None] * (HPC * NKSLAB)
                v_tiles = [None] * NGRP
                wp_sb = None

                def load_k(h, s_):
                    t_ = kp.tile([128, KSLAB], F8, tag="k", name=f"k{h}_{s_}{r}")
                    nc.sync.dma_start(t_[:], kt_d[h * NKSLAB + s_])
                    k_tiles[h * NKSLAB + s_] = t_

                def load_v(g):
                    t_ = vp.tile([128, GRP * HPC * VW], F8, tag="v",
                                 name=f"v{g}{r}")
                    nc.sync.dma_start(t_[:], vb_d[g])
                    v_tiles[g] = t_

                for s_ in range(NKSLAB):
                    load_k(0, s_)
                    load_k(1, s_)
                    for g in range(s_ * NGRP // NKSLAB,
                                   (s_ + 1) * NGRP // NKSLAB):
                        if g == NGRP - 2:
                            wp_sb = wqp.tile([128, HPC * HID], BF, tag="wp",
                                             name=f"wp{r}")
                            nc.sync.dma_start(wp_sb[:], wp_d[:])
                        load_v(g)

                # ---- qkv projection (transposed); m-pairs share one
                # [128,128] psum; DVE applies bias (+1/64 rescale for the
                # fp8-shipped k/v weights) ----
                qkvp = []      # [q_pair, k_pair, v_pair]: [:, h*64:+64] = head h
                for mp in range(3):
                    ps = pgpp.tile([128, 128], FP, tag="gp", name=f"qkvps{mp}{r}")
                    for half in range(2):
                        for t in range(16):
                            if mp == 0:
                                w_sl = wqq[:, half * 2048 + t * 128:half * 2048 + (t + 1) * 128]
                            else:
                                m2 = (mp - 1) * 2 + half
                                w_sl = wqkv[:, m2 * 2048 + t * 128:m2 * 2048 + (t + 1) * 128]
                            nc.tensor.matmul(
                                ps[:, half * 64:(half + 1) * 64], lhsT=w_sl,
                                rhs=xt[:, t * NQ:(t + 1) * NQ],
                                start=(t == 0), stop=(t == 15))
                    sb = constp.tile([128, 128], BF, tag=f"qkvp{mp}", name=f"qkvp{mp}{r}")
                    if mp == 0:
                        nc.vector.tensor_add(sb[:], ps[:], bqw[:, 0:128])
                    else:
                        nc.vector.scalar_tensor_tensor(
                            sb[:], ps[:], 1.0 / WS, bqw[:, mp * 128:(mp + 1) * 128],
                            ALU.mult, ALU.add)
                    qkvp.append(sb)
                qp, kp_, vp_ = qkvp

                # ---- new-token attention pieces (head h in rows h*64:+64) ----
                # one full transpose: vp_^T rows 0-63 = v_h0 (token-major),
                # rows 64-127 = v_h1
                vt_ps = pmiscp.tile([128, 128], BF, tag="misc", name=f"vtps{r}")
                nc.tensor.transpose(vt_ps[:], vp_[:], ident[:])
                vn = constp.tile([128, VW], BF, tag="vnew", name=f"vnew{r}")
                nc.vector.tensor_copy(vn[:, 0:128], vt_ps[:])
                nc.vector.memset(vn[:, 128:129], 1.0)
                sn_ps = pmiscp.tile([128, NQ], FP, tag="misc", name=f"snps{r}")
                for h in range(HPC):
                    nc.tensor.matmul(sn_ps[h * 64:(h + 1) * 64, :],
                                     lhsT=kp_[:, h * 64:(h + 1) * 64],
                                     rhs=qp[:, h * 64:(h + 1) * 64],
                                     start=True, stop=True)
                pn = constp.tile([128, NQ], BF, tag="pn", name=f"pn{r}")
                nc.scalar.activation(pn[:], sn_ps[:], AFT.Exp)
                pnm = constp.tile([128, NQ], BF, tag="pnm", name=f"pnm{r}")
                nc.vector.tensor_mul(pnm[:], pn[:], msk[:])

                # ---- cache sweep: both heads interleaved; the per-head M=64
                # attn@V matmuls pair into PE column groups ----
                acc = paccp.tile([128, VW], FP, tag="acc", name=f"acc{r}")
                for h in range(HPC):
                    # composed row+col tile position (64,64) for head 1: the
                    # 64-key contraction reads partitions h*64.., the output
                    # lands in PSUM rows h*64..
                    nc.tensor.matmul(acc[h * 64:(h + 1) * 64, :],
                                     lhsT=pnm[h * 64:(h + 1) * 64, :],
                                     rhs=vn[h * 64:(h + 1) * 64, :],
                                     start=True, stop=False,
                                     skip_group_check=True)

                def flush_acc(pending, last):
                    p_sb, g = pending
                    v_sb = v_tiles[g]
                    for c2 in range(GRP):
                        for h in range(HPC):
                            nc.tensor.matmul(
                                acc[h * 64:(h + 1) * 64, :],
                                lhsT=p_sb[:, (c2 * HPC + h) * NQ:(c2 * HPC + h + 1) * NQ],
                                rhs=v_sb[:, (c2 * HPC + h) * VW:(c2 * HPC + h + 1) * VW],
                                start=False,
                                stop=(last and c2 == GRP - 1),
                                skip_group_check=True)

                pending = None
                for g in range(NGRP):
                    s_ps = pssp.tile([128, GRP * HPC * NQ], FP, tag="s",
                                     name=f"s{g}{r}")
                    for c2 in range(GRP):
                        c = g * GRP + c2
                        koff = (c % (KSLAB // 128)) * 128
                        slab = c // (KSLAB // 128)
                        for h in range(HPC):
                            nc.tensor.matmul(
                                s_ps[:, (c2 * HPC + h) * NQ:(c2 * HPC + h + 1) * NQ],
                                lhsT=k_tiles[h * NKSLAB + slab][:, koff:koff + 128],
                                rhs=qp[:, h * 64:(h + 1) * 64],
                                start=True, stop=True)
                    p_sb = ptp.tile([128, GRP * HPC * NQ], BF, tag="pt",
                                    name=f"p{g}{r}")
                    nc.scalar.activation(p_sb[:], s_ps[:], AFT.Exp)
                    if pending is not None:
                        flush_acc(pending, False)
                    pending = (p_sb, g)
                flush_acc(pending, True)

                if DEBUG_TAPS:
                    dbg_acc_sb = smallp.tile([128, VW], FP, tag="dbga", name=f"dbga{r}")
                    nc.vector.tensor_copy(dbg_acc_sb[:], acc[:])
                    nc.sync.dma_start(dbg_acc_d[:], dbg_acc_sb[:])
                    dbg_q_sb = smallp.tile([128, 3 * 128], FP, tag="dbgq", name=f"dbgq{r}")
                    for mp in range(3):
                        nc.vector.tensor_copy(dbg_q_sb[:, mp * 128:(mp + 1) * 128], qkvp[mp][:])
                    nc.sync.dma_start(dbg_q_d[:], dbg_q_sb[:])
                    dbg_p_sb = smallp.tile([128, 1024], FP, tag="dbgp", name=f"dbgp{r}")
                    nc.vector.tensor_copy(dbg_p_sb[:], pending[0][:])
                    nc.sync.dma_start(dbg_p_d[:], dbg_p_sb[:])

                # ---- normalize + transpose (both heads at once) ----
                rec = smallp.tile([128, 1], FP, tag="rec", name=f"rec{r}")
                nc.vector.reciprocal(rec[:], acc[:, 128:129])
                u2 = smallp.tile([128, 128], BF, tag="u", name=f"u{r}")
                nc.vector.tensor_scalar_mul(u2[:], acc[:, 0:128], rec[:])
                # one full transpose: u2^T cols 0-63 = ut_h0, cols 64-127 = ut_h1
                ut_ps = pmiscp.tile([128, 128], BF, tag="misc", name=f"utps{r}")
                nc.tensor.transpose(ut_ps[:], u2[:], ident[:])
                ut = smallp.tile([128, 128], BF, tag="ut", name=f"ut{r}")
                nc.vector.tensor_copy(ut[:], ut_ps[:])

                # ---- output projection: 512-col blocks 2n/2n+1 pair into
                # PSUM rows 0-63 / 64-127 ----
                y_sb = smallp.tile([128, 1024], BF, tag="y_sb", name=f"y{r}")
                for np_ in range(2):
                    y_ps = pgpp.tile([128, 512], FP, tag="gp", name=f"yps{np_}{r}")
                    for half in range(2):
                        n = np_ * 2 + half
                        for h in range(HPC):
                            nc.tensor.matmul(
                                y_ps[half * 64:(half + 1) * 64, :],
                                lhsT=ut[:, h * 64:(h + 1) * 64],
                                rhs=wp_sb[:, h * HID + n * 512:h * HID + (n + 1) * 512],
                                start=(h == 0), stop=(h == HPC - 1))
                    nc.vector.tensor_copy(y_sb[:, np_ * 512:(np_ + 1) * 512], y_ps[:])
                    for half in range(2):
                        n = np_ * 2 + half
                        nc.sync.dma_start(
                            out_d[:, n * 512:(n + 1) * 512],
                            y_sb[half * 64:(half + 1) * 64, np_ * 512:(np_ + 1) * 512])

            if loop is None:
                for rep in range(reps):
                    emit(f"r{rep}")
            else:
                with tc.For_i(0, loop, 1,
                              hint_engines=(mybir.EngineType.PE,)):
                    emit("rl")

    nc.compile()
    return nc


def _prep_shards(x, cached_k, cached_v, w_qkv, b_qkv, w_proj):
    scale = np.float32(1.0 / math.sqrt(D))
    x2d = np.asarray(x, np.float32).reshape(NQ, HID)
    xt_host = np.ascontiguousarray(
        x2d.T.reshape(16, 128, NQ).transpose(1, 0, 2).reshape(128, 16 * NQ)
    ).astype(NPBF)
    mask = np.kron(np.eye(B, dtype=np.float32), np.ones((S, S), np.float32))
    mask2 = np.ascontiguousarray(np.concatenate([mask, mask], 0)).astype(NPBF)

    ck = np.asarray(cached_k, np.float32)
    cv = np.asarray(cached_v, np.float32)
    wq = np.asarray(w_qkv, np.float32)
    bq = np.asarray(b_qkv, np.float32)
    wp = np.asarray(w_proj, np.float32)

    in_maps = []
    for core in range(NCORES):
        h0 = HPC * core
        cols = slice(h0 * D, (h0 + HPC) * D)
        wq_q = wq[:, 0:HID][:, cols] * scale                     # [2048, 256]
        wqq_host = np.ascontiguousarray(
            wq_q.reshape(16, 128, 2, 128).transpose(1, 2, 0, 3).reshape(128, 2 * 2048)
        ).astype(NPBF)
        wq_kv = np.concatenate(
            [wq[:, HID:2 * HID][:, cols], wq[:, 2 * HID:3 * HID][:, cols]],
            axis=1) * WS                                          # [2048, 512]
        wqkv_host = np.ascontiguousarray(
            wq_kv.reshape(16, 128, 4, 128).transpose(1, 2, 0, 3).reshape(128, 4 * 2048)
        ).astype(NPF8)
        # bias pairs broadcast along the 64-token free dim: [q0|q1|k0|k1|v0|v1]
        b_shard = np.stack(
            [bq[0:HID][cols][i * 128:(i + 1) * 128] * scale if i < 2 else
             np.concatenate([bq[HID:2 * HID][cols], bq[2 * HID:3 * HID][cols]]
                            )[(i - 2) * 128:(i - 1) * 128]
             for i in range(6)])                                  # [6, 128]
        bqw_host = np.ascontiguousarray(
            np.repeat(b_shard[:, :, None], NQ, axis=2)            # [6,128,64]
              .reshape(3, 2, 128, NQ).transpose(2, 0, 1, 3).reshape(128, 3 * 128)
        ).astype(NPBF)

        kt_slabs = []
        for h in (h0, h0 + 1):
            kt_h = ck[:, h, :].T                                 # [128, 8192]
            kt_slabs.append(kt_h.reshape(128, NKSLAB, KSLAB).transpose(1, 0, 2))
        kt_host = np.ascontiguousarray(np.concatenate(kt_slabs, axis=0)).astype(NPF8)

        vb = np.empty((P, HPC * VW), np.float32)
        vb[:, 0:D] = cv[:, h0, :]
        vb[:, D] = 1.0
        vb[:, VW:VW + D] = cv[:, h0 + 1, :]
        vb[:, VW + D] = 1.0
        vb_host = np.ascontiguousarray(
            vb.reshape(NGRP, GRP, 128, HPC * VW)
              .transpose(0, 2, 1, 3).reshape(NGRP, 128, GRP * HPC * VW)
        ).astype(NPF8)

        wp_host = np.ascontiguousarray(
            np.concatenate([wp[(h0 + h) * D:(h0 + h + 1) * D, :]
                            for h in range(HPC)], axis=1)).astype(NPBF)

        in_maps.append({
            "xt": xt_host, "wqq": wqq_host, "wqkv": wqkv_host,
            "bqw": bqw_host, "mask": mask2,
            "kt": kt_host, "vb": vb_host, "wp": wp_host,
        })
    return in_maps


def kernel(**inputs):
    global _nc_cache
    x = np.asarray(inputs["x"], np.float32)
    b_proj = np.asarray(inputs["b_proj"], np.float32)
    in_maps = _prep_shards(
        x, inputs["cached_k"], inputs["cached_v"],
        inputs["w_qkv"], inputs["b_qkv"], inputs["w_proj"],
    )
    if _nc_cache is None:
        _nc_cache = _build_nc()
    res = run_bass_kernel_spmd(_nc_cache, in_maps, core_ids=list(range(NCORES)))
    y = np.zeros((NQ, HID), np.float64)
    for r in res.results:
        y += r["out"].astype(np.float64)
    y += b_proj.astype(np.float64)
    return y.astype(np.float32).reshape(B, S, HID)


# revision 15
# speedup vs baseline: 2.1192x; 1.2368x over previous
"""Trainium2 Bass kernel for nn_AttentionLayer (sparse_attention, 8-core head-parallel).

Reference computation (B=4, S=16, H=16, D=128, HID=2048, P=8192):
    qkv = x @ w_qkv + b_qkv ; split into q,k,v
    k_full = concat(cached_k broadcast over batch, new k)   # [B,H,P+S,D]
    out = softmax(q @ k_full^T / sqrt(D)) @ v_full
    y = out @ w_proj + b_proj

Sharding: tensor-parallel over heads. Each of the 8 cores owns 2 heads:
column-sharded w_qkv/b_qkv, the head slice of the KV cache, and the row slice
of w_proj. Each core emits a partial y [64, 2048] (bf16); the unshard step
sums the 8 partials and adds b_proj.

Numerics (numpy-simulated against the exact reference data; bf16 sim matched
HW to 4 digits): K cache, V cache and the k/v column blocks of w_qkv ship as
fp8 E3M4 (~1.2e-2 end-to-end rel err vs 2e-2 tolerance). q columns and w_proj
stay bf16. w_kv is stored x64 (clears E3M4's subnormal floor); the 1/64 folds
into the DVE bias op.

Performance structure (the kernel is bound by the PE LDWEIGHTS port and the
ACT engine, not DMA):
  - Transposed dataflow: qkv^T from the projection, scores^T per 128-key
    chunk (stationary = fp8 K^T tile, FWL-eligible), exp on [128,1024] PSUM
    tiles (one ACT instr per 16 chunk-scores), attn@V accumulated into one
    [128,129] PSUM tile.
  - The two heads' M=64 matmuls (attn@V, new-token pieces, proj blocks) are
    paired into different PE column groups via base-partition placement
    (tile_position auto-derives), so each pair runs concurrently: head 0 in
    PSUM rows 0-63, head 1 in rows 64-127.
  - exp(scores) for group g+1 is issued before attn@V of group g so the PE
    never waits on the ACT engine.
  - All non-transcendental elementwise work (qkv bias+rescale, normalize,
    PSUM->SBUF copies) runs on the otherwise idle DVE, not ACT.
  - V ships per chunk as [v_h0 | 1 | v_h1 | 1] so exp(scores^T)^T @ [V | 1]
    yields numerator and softmax denominator in one accumulation (scores are
    O(5): exp needs no max-subtraction).
  - All input DMAs are issued up front in consumption order.
"""

import math

import numpy as np
import ml_dtypes

import concourse.bass as bass
import concourse.mybir as mybir
import concourse.tile as tile
from concourse import bacc
from concourse.bass_utils import run_bass_kernel_spmd
from concourse.masks import make_identity

FP = mybir.dt.float32
BF = mybir.dt.bfloat16
F8 = mybir.dt.float8e3
NPBF = ml_dtypes.bfloat16
NPF8 = ml_dtypes.float8_e3m4
AFT = mybir.ActivationFunctionType
ALU = mybir.AluOpType

B, S, H, D = 4, 16, 16, 128
HID = H * D            # 2048
P = 8192               # cached prefix length
NQ = B * S             # 64 query tokens
NCORES = 8
HPC = H // NCORES      # heads per core = 2

NCHUNK = P // 128      # 64 cache chunks of 128 keys per head
GRP = 8                # chunks (both heads) per exp group -> [128,1024] PSUM
NGRP = NCHUNK // GRP   # 8 groups
KSLAB = 4096           # keys per K-slab DMA (512KB fp8)
NKSLAB = P // KSLAB    # 2 slabs per head
VW = D + 1             # 129: V columns + ones column
WS = 64.0              # fp8 weight prescale

_nc_cache = None
DEBUG_TAPS = False


def _build_nc(reps=1, loop=None, unroll=1):
    nc = bacc.Bacc("TRN2", target_bir_lowering=False, debug=False,
                   num_devices=NCORES)

    xt_d = nc.declare_dram_parameter("xt", [128, 16 * NQ], BF, isOutput=False)
    wqq_d = nc.declare_dram_parameter("wqq", [128, 2 * 2048], BF, isOutput=False)
    wqkv_d = nc.declare_dram_parameter("wqkv", [128, 4 * 2048], F8, isOutput=False)
    bqw_d = nc.declare_dram_parameter("bqw", [128, 3 * 128], BF, isOutput=False)
    mask_d = nc.declare_dram_parameter("mask", [128, NQ], BF, isOutput=False)
    kt_d = nc.declare_dram_parameter("kt", [HPC * NKSLAB, 128, KSLAB], F8, isOutput=False)
    vb_d = nc.declare_dram_parameter("vb", [NGRP, 128, GRP * HPC * VW], F8, isOutput=False)
    wp_d = nc.declare_dram_parameter("wp", [128, HPC * HID], BF, isOutput=False)
    out_d = nc.declare_dram_parameter("out", [NQ, HID], BF, isOutput=True)
    if DEBUG_TAPS:
        dbg_q_d = nc.declare_dram_parameter("dbg_q", [128, 3 * 128], FP, isOutput=True)
        dbg_acc_d = nc.declare_dram_parameter("dbg_acc", [128, VW], FP, isOutput=True)
        dbg_p_d = nc.declare_dram_parameter("dbg_p", [128, 1024], FP, isOutput=True)

    with tile.TileContext(nc) as tc:
        with (
            tc.tile_pool(name="const", bufs=1) as constp,
            tc.tile_pool(name="weights", bufs=1) as wqp,
            tc.tile_pool(name="kslab", bufs=HPC * NKSLAB) as kp,
            tc.tile_pool(name="vslab", bufs=NGRP) as vp,
            tc.tile_pool(name="pt", bufs=3) as ptp,
            tc.tile_pool(name="small", bufs=4) as smallp,
            tc.tile_pool(name="ps_s", bufs=2, space="PSUM") as pssp,
            tc.tile_pool(name="ps_acc", bufs=1, space="PSUM") as paccp,
            tc.tile_pool(name="ps_gp", bufs=2, space="PSUM") as pgpp,
            tc.tile_pool(name="ps_misc", bufs=1, space="PSUM") as pmiscp,
        ):
            ident = constp.tile([128, 128], BF, tag="ident")
            make_identity(nc, ident[:])

            def emit(r):
                # ---- the whole input stream, issued up front in
                # consumption order ----
                xt = constp.tile([128, 16 * NQ], BF, tag="xt", name=f"xt{r}")
                nc.sync.dma_start(xt[:], xt_d[:])
                bqw = constp.tile([128, 3 * 128], BF, tag="bqw", name=f"bqw{r}")
                nc.sync.dma_start(bqw[:], bqw_d[:])
                msk = constp.tile([128, NQ], BF, tag="msk", name=f"msk{r}")
                nc.sync.dma_start(msk[:], mask_d[:])
                wqq = wqp.tile([128, 2 * 2048], BF, tag="wqq", name=f"wqq{r}")
                nc.sync.dma_start(wqq[:], wqq_d[:])
                wqkv = wqp.tile([128, 4 * 2048], F8, tag="wqkv", name=f"wqkv{r}")
                nc.sync.dma_start(wqkv[:], wqkv_d[:])

                k_tiles = [None] * (HPC * NKSLAB)
                v_tiles = [None] * NGRP
                wp_sb = None

                def load_k(h, s_):
                    t_ = kp.tile([128, KSLAB], F8, tag="k", name=f"k{h}_{s_}{r}")
                    nc.sync.dma_start(t_[:], kt_d[h * NKSLAB + s_])
                    k_tiles[h * NKSLAB + s_] = t_

                def load_v(g):
                    t_ = vp.tile([128, GRP * HPC * VW], F8, tag="v",
                                 name=f"v{g}{r}")
                    nc.sync.dma_start(t_[:], vb_d[g])
                    v_tiles[g] = t_

                for s_ in range(NKSLAB):
                    load_k(0, s_)
                    load_k(1, s_)
                    for g in range(s_ * NGRP // NKSLAB,
                                   (s_ + 1) * NGRP // NKSLAB):
                        if g == NGRP - 2:
                            wp_sb = wqp.tile([128, HPC * HID], BF, tag="wp",
                                             name=f"wp{r}")
                            nc.sync.dma_start(wp_sb[:], wp_d[:])
                        load_v(g)

                # ---- qkv projection (transposed); m-pairs share one
                # [128,128] psum; DVE applies bias (+1/64 rescale for the
                # fp8-shipped k/v weights) ----
                qkvp = []      # [q_pair, k_pair, v_pair]: [:, h*64:+64] = head h
                for mp in range(3):
                    ps = pgpp.tile([128, 128], FP, tag="gp", name=f"qkvps{mp}{r}")
                    for half in range(2):
                        for t in range(16):
                            if mp == 0:
                                w_sl = wqq[:, half * 2048 + t * 128:half * 2048 + (t + 1) * 128]
                            else:
                                m2 = (mp - 1) * 2 + half
                                w_sl = wqkv[:, m2 * 2048 + t * 128:m2 * 2048 + (t + 1) * 128]
                            nc.tensor.matmul(
                                ps[:, half * 64:(half + 1) * 64], lhsT=w_sl,
                                rhs=xt[:, t * NQ:(t + 1) * NQ],
                                start=(t == 0), stop=(t == 15))
                    sb = constp.tile([128, 128], BF, tag=f"qkvp{mp}", name=f"qkvp{mp}{r}")
                    if mp == 0:
                        nc.vector.tensor_add(sb[:], ps[:], bqw[:, 0:128])
                    else:
                        nc.vector.scalar_tensor_tensor(
                            sb[:], ps[:], 1.0 / WS, bqw[:, mp * 128:(mp + 1) * 128],
                            ALU.mult, ALU.add)
                    qkvp.append(sb)
                qp, kp_, vp_ = qkvp

                # ---- new-token attention pieces (head h in rows h*64:+64) ----
                # one full transpose: vp_^T rows 0-63 = v_h0 (token-major),
                # rows 64-127 = v_h1
                vt_ps = pmiscp.tile([128, 128], BF, tag="misc", name=f"vtps{r}")
                nc.tensor.transpose(vt_ps[:], vp_[:], ident[:])
                vn = constp.tile([128, VW], BF, tag="vnew", name=f"vnew{r}")
                nc.vector.tensor_copy(vn[:, 0:128], vt_ps[:])
                nc.vector.memset(vn[:, 128:129], 1.0)
                sn_ps = pmiscp.tile([128, NQ], FP, tag="misc", name=f"snps{r}")
                for h in range(HPC):
                    nc.tensor.matmul(sn_ps[h * 64:(h + 1) * 64, :],
                                     lhsT=kp_[:, h * 64:(h + 1) * 64],
                                     rhs=qp[:, h * 64:(h + 1) * 64],
                                     start=True, stop=True)
                pn = constp.tile([128, NQ], BF, tag="pn", name=f"pn{r}")
                nc.scalar.activation(pn[:], sn_ps[:], AFT.Exp)
                pnm = constp.tile([128, NQ], BF, tag="pnm", name=f"pnm{r}")
                nc.vector.tensor_mul(pnm[:], pn[:], msk[:])

                # ---- cache sweep: both heads interleaved; the per-head M=64
                # attn@V matmuls pair into PE column groups ----
                acc = paccp.tile([128, VW], FP, tag="acc", name=f"acc{r}")
                for h in range(HPC):
                    # composed row+col tile position (64,64) for head 1: the
                    # 64-key contraction reads partitions h*64.., the output
                    # lands in PSUM rows h*64..
                    nc.tensor.matmul(acc[h * 64:(h + 1) * 64, :],
                                     lhsT=pnm[h * 64:(h + 1) * 64, :],
                                     rhs=vn[h * 64:(h + 1) * 64, :],
                                     start=True, stop=False,
                                     skip_group_check=True)

                def flush_acc(pending, last):
                    p_sb, g = pending
                    v_sb = v_tiles[g]
                    for c2 in range(GRP):
                        for h in range(HPC):
                            nc.tensor.matmul(
                                acc[h * 64:(h + 1) * 64, :],
                                lhsT=p_sb[:, (c2 * HPC + h) * NQ:(c2 * HPC + h + 1) * NQ],
                                rhs=v_sb[:, (c2 * HPC + h) * VW:(c2 * HPC + h + 1) * VW],
                                start=False,
                                stop=(last and c2 == GRP - 1),
                                skip_group_check=True)

                pending = None
                for g in range(NGRP):
                    s_ps = pssp.tile([128, GRP * HPC * NQ], FP, tag="s",
                                     name=f"s{g}{r}")
                    for c2 in range(GRP):
                        c = g * GRP + c2
                        koff = (c % (KSLAB // 128)) * 128
                        slab = c // (KSLAB // 128)
                        for h in range(HPC):
                            nc.tensor.matmul(
                                s_ps[:, (c2 * HPC + h) * NQ:(c2 * HPC + h + 1) * NQ],
                                lhsT=k_tiles[h * NKSLAB + slab][:, koff:koff + 128],
                                rhs=qp[:, h * 64:(h + 1) * 64],
                                start=True, stop=True)
                    p_sb = ptp.tile([128, GRP * HPC * NQ], BF, tag="pt",
                                    name=f"p{g}{r}")
                    nc.scalar.activation(p_sb[:], s_ps[:], AFT.Exp)
                    if pending is not None:
                        flush_acc(pending, False)
                    pending = (p_sb, g)
                flush_acc(pending, True)

                if DEBUG_TAPS:
                    dbg_acc_sb = smallp.tile([128, VW], FP, tag="dbga", name=f"dbga{r}")
                    nc.vector.tensor_copy(dbg_acc_sb[:], acc[:])
                    nc.sync.dma_start(dbg_acc_d[:], dbg_acc_sb[:])
                    dbg_q_sb = smallp.tile([128, 3 * 128], FP, tag="dbgq", name=f"dbgq{r}")
                    for mp in range(3):
                        nc.vector.tensor_copy(dbg_q_sb[:, mp * 128:(mp + 1) * 128], qkvp[mp][:])
                    nc.sync.dma_start(dbg_q_d[:], dbg_q_sb[:])
                    dbg_p_sb = smallp.tile([128, 1024], FP, tag="dbgp", name=f"dbgp{r}")
                    nc.vector.tensor_copy(dbg_p_sb[:], pending[0][:])
                    nc.sync.dma_start(dbg_p_d[:], dbg_p_sb[:])

                # ---- normalize + transpose (both heads at once) ----
                rec = smallp.tile([128, 1], FP, tag="rec", name=f"rec{r}")
                nc.vector.reciprocal(rec[:], acc[:, 128:129])
                u2 = smallp.tile([128, 128], BF, tag="u", name=f"u{r}")
                nc.vector.tensor_scalar_mul(u2[:], acc[:, 0:128], rec[:])
                # one full transpose: u2^T cols 0-63 = ut_h0, cols 64-127 = ut_h1
                ut_ps = pmiscp.tile([128, 128], BF, tag="misc", name=f"utps{r}")
                nc.tensor.transpose(ut_ps[:], u2[:], ident[:])
                ut = smallp.tile([128, 128], BF, tag="ut", name=f"ut{r}")
                nc.vector.tensor_copy(ut[:], ut_ps[:])

                # ---- output projection: 512-col blocks 2n/2n+1 pair into
                # PSUM rows 0-63 / 64-127 ----
                y_sb = smallp.tile([128, 1024], BF, tag="y_sb", name=f"y{r}")
                for np_ in range(2):
                    y_ps = pgpp.tile([128, 512], FP, tag="gp", name=f"yps{np_}{r}")
                    for half in range(2):
                        n = np_ * 2 + half
                        for h in range(HPC):
                            nc.tensor.matmul(
                                y_ps[half * 64:(half + 1) * 64, :],
                                lhsT=ut[:, h * 64:(h + 1) * 64],
                                rhs=wp_sb[:, h * HID + n * 512:h * HID + (n + 1) * 512],
                                start=(h == 0), stop=(h == HPC - 1))
                    nc.vector.tensor_copy(y_sb[:, np_ * 512:(np_ + 1) * 512], y_ps[:])
                    for half in range(2):
                        n = np_ * 2 + half
                        nc.sync.dma_start(
                            out_d[:, n * 512:(n + 1) * 512],
                            y_sb[half * 64:(half + 1) * 64, np_ * 512:(np_ + 1) * 512])

            if loop is None:
                for rep in range(reps):
                    emit(f"r{rep}")
            else:
                with tc.For_i(0, loop, 1,
                              hint_engines=(mybir.EngineType.PE,)):
                    for u in range(unroll):
                        emit(f"u{u}")

    nc.compile()
    return nc


def _prep_shards(x, cached_k, cached_v, w_qkv, b_qkv, w_proj):
    scale = np.float32(1.0 / math.sqrt(D))
    x2d = np.asarray(x, np.float32).reshape(NQ, HID)
    xt_host = np.ascontiguousarray(
        x2d.T.reshape(16, 128, NQ).transpose(1, 0, 2).reshape(128, 16 * NQ)
    ).astype(NPBF)
    mask = np.kron(np.eye(B, dtype=np.float32), np.ones((S, S), np.float32))
    mask2 = np.ascontiguousarray(np.concatenate([mask, mask], 0)).astype(NPBF)

    ck = np.asarray(cached_k, np.float32)
    cv = np.asarray(cached_v, np.float32)
    wq = np.asarray(w_qkv, np.float32)
    bq = np.asarray(b_qkv, np.float32)
    wp = np.asarray(w_proj, np.float32)

    in_maps = []
    for core in range(NCORES):
        h0 = HPC * core
        cols = slice(h0 * D, (h0 + HPC) * D)
        wq_q = wq[:, 0:HID][:, cols] * scale                     # [2048, 256]
        wqq_host = np.ascontiguousarray(
            wq_q.reshape(16, 128, 2, 128).transpose(1, 2, 0, 3).reshape(128, 2 * 2048)
        ).astype(NPBF)
        wq_kv = np.concatenate(
            [wq[:, HID:2 * HID][:, cols], wq[:, 2 * HID:3 * HID][:, cols]],
            axis=1) * WS                                          # [2048, 512]
        wqkv_host = np.ascontiguousarray(
            wq_kv.reshape(16, 128, 4, 128).transpose(1, 2, 0, 3).reshape(128, 4 * 2048)
        ).astype(NPF8)
        # bias pairs broadcast along the 64-token free dim: [q0|q1|k0|k1|v0|v1]
        b_shard = np.stack(
            [bq[0:HID][cols][i * 128:(i + 1) * 128] * scale if i < 2 else
             np.concatenate([bq[HID:2 * HID][cols], bq[2 * HID:3 * HID][cols]]
                            )[(i - 2) * 128:(i - 1) * 128]
             for i in range(6)])                                  # [6, 128]
        bqw_host = np.ascontiguousarray(
            np.repeat(b_shard[:, :, None], NQ, axis=2)            # [6,128,64]
              .reshape(3, 2, 128, NQ).transpose(2, 0, 1, 3).reshape(128, 3 * 128)
        ).astype(NPBF)

        kt_slabs = []
        for h in (h0, h0 + 1):
            kt_h = ck[:, h, :].T                                 # [128, 8192]
            kt_slabs.append(kt_h.reshape(128, NKSLAB, KSLAB).transpose(1, 0, 2))
        kt_host = np.ascontiguousarray(np.concatenate(kt_slabs, axis=0)).astype(NPF8)

        vb = np.empty((P, HPC * VW), np.float32)
        vb[:, 0:D] = cv[:, h0, :]
        vb[:, D] = 1.0
        vb[:, VW:VW + D] = cv[:, h0 + 1, :]
        vb[:, VW + D] = 1.0
        vb_host = np.ascontiguousarray(
            vb.reshape(NGRP, GRP, 128, HPC * VW)
              .transpose(0, 2, 1, 3).reshape(NGRP, 128, GRP * HPC * VW)
        ).astype(NPF8)

        wp_host = np.ascontiguousarray(
            np.concatenate([wp[(h0 + h) * D:(h0 + h + 1) * D, :]
                            for h in range(HPC)], axis=1)).astype(NPBF)

        in_maps.append({
            "xt": xt_host, "wqq": wqq_host, "wqkv": wqkv_host,
            "bqw": bqw_host, "mask": mask2,
            "kt": kt_host, "vb": vb_host, "wp": wp_host,
        })
    return in_maps


def kernel(**inputs):
    global _nc_cache
    x = np.asarray(inputs["x"], np.float32)
    b_proj = np.asarray(inputs["b_proj"], np.float32)
    in_maps = _prep_shards(
        x, inputs["cached_k"], inputs["cached_v"],
        inputs["w_qkv"], inputs["b_qkv"], inputs["w_proj"],
    )
    if _nc_cache is None:
        _nc_cache = _build_nc()
    res = run_bass_kernel_spmd(_nc_cache, in_maps, core_ids=list(range(NCORES)))
    y = np.zeros((NQ, HID), np.float64)
    for r in res.results:
        y += r["out"].astype(np.float64)
    y += b_proj.astype(np.float64)
    return y.astype(np.float32).reshape(B, S, HID)
